# revision 1
# baseline (speedup 1.0000x reference)
"""NF4-quantized linear + LoRA kernel for Trainium2 (Bass/Tile), 8-core SPMD.

Contract: kernel(**inputs) takes the FULL unsharded inputs
    x      [4096, 4096] float32
    codes  [4096, 4096] int32   (NF4 code indices, 0..15)
    scales [262144]     float32 (one absmax scale per 64 contiguous elements)
    lora_A [16, 4096]   float32
    lora_B [4096, 16]   float32
and returns the full output  y = x @ dequant(codes, scales).T + (x @ A.T) @ B.T * 2.0
of shape [4096, 4096] float32.

Sharding: tensor-parallel over out_features (column parallel). Each of the 8
NeuronCores gets codes/scales/lora_B rows for its 512 output columns plus a
full replica of x and lora_A, computes y_shard [4096, 512] on device, and the
shards are concatenated on the host.

Device algorithm per core:
  1. Dequantize the W shard on-chip: extract the 4 bits of each code with
     is_ge/multiply-add chains (VectorE), build the 8 pair-leaf affine values
     T[2j] + b0*(T[2j+1]-T[2j]) (split across ScalarE/GpSimd/VectorE), then a
     binary select tree of copy_predicated ops (VectorE) yields NF4[c] exactly
     in bf16; multiply by the block scales (broadcast access pattern).
  2. Fold the LoRA correction in as W' = W + 2.0 * (B @ A) via a tiny K=16
     matmul on the TensorEngine accumulated before the transpose.
  3. PE-transpose W' into a resident [i, o]-chunked bf16 copy in SBUF.
  4. Stream x: DMA fp32 tiles, cast to bf16, PE-transpose (contraction dim to
     partitions), then 32 accumulating K=128 matmuls per 128-token tile into
     PSUM; copy out via ScalarE and DMA to HBM.
"""
import numpy as np

import concourse.bass as bass
import concourse.bacc as bacc
import concourse.mybir as mybir
import concourse.tile as tile
from concourse.bass_utils import run_bass_kernel_spmd
from concourse.masks import make_identity

dt = mybir.dt
A_ = mybir.AluOpType

NF4 = np.array([-1.0, -0.6961928009986877, -0.5250730514526367, -0.39491748809814453,
                -0.28444138169288635, -0.18477343022823334, -0.09105003625154495, 0.0,
                0.07958029955625534, 0.16093020141124725, 0.24611230194568634,
                0.33791524171829224, 0.44070982933044434, 0.5626170039176941,
                0.7229568362236023, 1.0], dtype=np.float32)

N_CORES = 8
T, IN, OUT, R = 4096, 4096, 4096, 16
O = OUT // N_CORES          # 512 out columns per core
BLK = 64                    # quant block size
SCALING = 2.0               # lora_alpha / r
N_OT = O // 128             # o-tiles per core
N_IC = IN // 128            # contraction chunks
SUB = 1024                  # dequant sub-tile width
N_SUB = IN // SUB


def _build(nc):
    x_d = nc.dram_tensor("x", [T, IN], dt.float32, kind="ExternalInput").ap()
    codes_d = nc.dram_tensor("codes", [O, IN], dt.int32, kind="ExternalInput").ap()
    scales_d = nc.dram_tensor("scales", [O, IN // BLK], dt.float32, kind="ExternalInput").ap()
    lora_a_d = nc.dram_tensor("lora_a", [R, IN], dt.float32, kind="ExternalInput").ap()
    lora_b_d = nc.dram_tensor("lora_b", [O, R], dt.float32, kind="ExternalInput").ap()
    y_d = nc.dram_tensor("y", [T, O], dt.float32, kind="ExternalOutput").ap()

    with tile.TileContext(nc) as tc:
        with tc.tile_pool(name="persist", bufs=1) as pp:
            wt = pp.tile([128, N_IC * O], dt.bfloat16, name="wt")
            ident = pp.tile([128, 128], dt.bfloat16, name="ident")
            make_identity(nc, ident)

            # ---- LoRA prep: a_bf [r, i] bf16, bt [r, o] bf16 (scaled by alpha/r) ----
            with tc.tile_pool(name="lora", bufs=1) as lp, \
                 tc.tile_pool(name="lpsum", bufs=2, space="PSUM") as lps:
                a_f = lp.tile([R, IN], dt.float32)
                nc.sync.dma_start(out=a_f, in_=lora_a_d)
                a_bf = pp.tile([R, IN], dt.bfloat16, name="a_bf")
                nc.scalar.copy(a_bf, a_f)
                b_f = lp.tile([128, N_OT * R], dt.float32)
                for b in range(N_OT):
                    nc.sync.dma_start(out=b_f[:, b * R:(b + 1) * R],
                                      in_=lora_b_d[b * 128:(b + 1) * 128, :])
                b_bf = lp.tile([128, N_OT * R], dt.bfloat16)
                nc.scalar.copy(b_bf, b_f)
                bt = pp.tile([R, O], dt.bfloat16, name="bt")
                for b in range(N_OT):
                    pst = lps.tile([R, 128], dt.bfloat16)
                    nc.tensor.transpose(pst, b_bf[:, b * R:(b + 1) * R], ident)
                    nc.scalar.activation(bt[:, b * 128:(b + 1) * 128], pst,
                                         mybir.ActivationFunctionType.Copy,
                                         scale=SCALING)

            with tc.tile_pool(name="wph", bufs=2) as wp, \
                 tc.tile_pool(name="wscr", bufs=1) as ws, \
                 tc.tile_pool(name="wpsum", bufs=2, space="PSUM") as wps, \
                 tc.tile_pool(name="mt", bufs=2) as mp, \
                 tc.tile_pool(name="mpsum", bufs=2, space="PSUM") as mps, \
                 tc.tile_pool(name="tpsum2", bufs=2, space="PSUM") as tps:

                # ---- W phase: dequant + lora-fold + transpose into wt ----
                scal = ws.tile([128, N_OT * (IN // BLK)], dt.bfloat16, name="scal")
                scal_f = ws.tile([128, N_OT * (IN // BLK)], dt.float32, name="scal_f")
                for b in range(N_OT):
                    nc.sync.dma_start(out=scal_f[:, b * 64:(b + 1) * 64],
                                      in_=scales_d[b * 128:(b + 1) * 128, :])
                nc.scalar.copy(scal, scal_f)

                for b in range(N_OT):
                    codes = wp.tile([128, IN // 2], dt.int32, tag="codes", bufs=2)
                    nc.sync.dma_start(out=codes,
                                      in_=codes_d[b * 128:(b + 1) * 128, 0:IN // 2])
                    codes2 = wp.tile([128, IN // 2], dt.int32, tag="codes2", bufs=2)
                    nc.sync.dma_start(out=codes2,
                                      in_=codes_d[b * 128:(b + 1) * 128, IN // 2:])
                    wpr = wp.tile([128, IN], dt.bfloat16, tag="wpr")

                    def emit_bits(s):
                        half = codes if s < N_SUB // 2 else codes2
                        s_in = s if s < N_SUB // 2 else s - N_SUB // 2
                        cs = ws.tile([128, SUB], dt.int16, tag="cs", bufs=2)
                        nc.scalar.copy(cs, half[:, s_in * SUB:(s_in + 1) * SUB])
                        b3 = ws.tile([128, SUB], dt.int16, tag="b3", bufs=2)
                        r3 = ws.tile([128, SUB], dt.int16, tag="r3")
                        b2 = ws.tile([128, SUB], dt.int16, tag="b2", bufs=2)
                        r2 = ws.tile([128, SUB], dt.int16, tag="r2")
                        b1 = ws.tile([128, SUB], dt.int16, tag="b1", bufs=2)
                        b0 = ws.tile([128, SUB], dt.int16, tag="b0", bufs=2)
                        nc.vector.tensor_scalar(b3, cs, 8, None, op0=A_.is_ge)
                        nc.vector.scalar_tensor_tensor(r3, b3, -8, cs, op0=A_.mult, op1=A_.add)
                        nc.vector.tensor_scalar(b2, r3, 4, None, op0=A_.is_ge)
                        nc.vector.scalar_tensor_tensor(r2, b2, -4, r3, op0=A_.mult, op1=A_.add)
                        nc.vector.tensor_scalar(b1, r2, 2, None, op0=A_.is_ge)
                        nc.vector.scalar_tensor_tensor(b0, b1, -2, r2, op0=A_.mult, op1=A_.add)
                        return b0, b1, b2, b3

                    def emit_leaves(bits):
                        b0 = bits[0]
                        leaves = []
                        for j in range(8):
                            lj = ws.tile([128, SUB], dt.bfloat16, tag=f"leaf{j}", bufs=2)
                            d = float(NF4[2 * j + 1] - NF4[2 * j])
                            t0 = float(NF4[2 * j])
                            if j < 2:
                                nc.scalar.activation(lj, b0,
                                                     mybir.ActivationFunctionType.Copy,
                                                     bias=t0, scale=d)
                            elif j < 6:
                                nc.gpsimd.tensor_scalar(lj, b0, d, t0,
                                                        op0=A_.mult, op1=A_.add)
                            else:
                                nc.vector.tensor_scalar(lj, b0, d, t0,
                                                        op0=A_.mult, op1=A_.add)
                            leaves.append(lj)
                        return leaves

                    def emit_tail(s, st):
                        leaves, (b0, b1, b2, b3) = st
                        for j in range(4):
                            nc.vector.copy_predicated(leaves[2 * j], b1, leaves[2 * j + 1])
                        nc.vector.copy_predicated(leaves[0], b2, leaves[2])
                        nc.vector.copy_predicated(leaves[4], b2, leaves[6])
                        nc.vector.copy_predicated(leaves[0], b3, leaves[4])
                        sexp = ws.tile([128, SUB], dt.bfloat16, tag="sexp", bufs=2)
                        sc = scal[:, b * 64 + s * (SUB // BLK): b * 64 + (s + 1) * (SUB // BLK)]
                        s_b = bass.AP(sc.tensor, sc.offset, [sc.ap[0], sc.ap[1], [0, BLK]])
                        nc.scalar.copy(sexp.rearrange("p (k j) -> p k j", j=BLK), s_b)
                        sl = slice(s * SUB, (s + 1) * SUB)
                        nc.vector.tensor_tensor(wpr[:, sl], leaves[0], sexp, op=A_.mult)

                    prev = None
                    for s in range(N_SUB):
                        bits = emit_bits(s)
                        leaves = emit_leaves(bits)
                        if prev is not None:
                            emit_tail(s - 1, prev)
                        prev = (leaves, bits)
                    emit_tail(N_SUB - 1, prev)

                    # lora fold: wpr += SCALING * (B @ A) for this o-tile
                    for hb in range(2):
                        ba_bf = ws.tile([128, IN // 2], dt.bfloat16, tag="ba")
                        for s2 in range(IN // 1024):
                            s2g = hb * (IN // 1024) + s2
                            bap = wps.tile([128, 512], dt.float32, tag="bapsum")
                            nc.tensor.matmul(bap, bt[:, b * 128:(b + 1) * 128],
                                             a_bf[:, s2g * 512:(s2g + 1) * 512],
                                             start=True, stop=True)
                            nc.scalar.copy(ba_bf[:, s2 * 512:(s2 + 1) * 512], bap)
                        hs = slice(hb * (IN // 2), (hb + 1) * (IN // 2))
                        nc.vector.tensor_tensor(wpr[:, hs], wpr[:, hs], ba_bf, op=A_.add)

                    # transpose [o,i] -> wt chunks [i, o]
                    for g in range(N_IC // 8):
                        tp = wps.tile([128, 8 * 128], dt.bfloat16, tag="tpsum")
                        for k in range(8):
                            c = g * 8 + k
                            nc.tensor.transpose(tp[:, k * 128:(k + 1) * 128],
                                                wpr[:, c * 128:(c + 1) * 128], ident)
                        outap = bass.AP(wt.tensor, wt.offset + g * 8 * O + b * 128,
                                        [wt.ap[0], [O, 8], [1, 128]])
                        nc.scalar.copy(outap, tp.rearrange("p (k f) -> p k f", k=8))

                # ---- main loop over 128-token tiles ----
                def stage(it):
                    xbf = mp.tile([128, IN], dt.bfloat16, tag="xbf")
                    for hf in range(2):
                        xf = mp.tile([128, IN // 2], dt.float32, tag="xf", bufs=2)
                        nc.sync.dma_start(out=xf, in_=x_d[it * 128:(it + 1) * 128,
                                                          hf * (IN // 2):(hf + 1) * (IN // 2)])
                        nc.vector.tensor_copy(xbf[:, hf * (IN // 2):(hf + 1) * (IN // 2)], xf)
                    xt = mp.tile([128, IN], dt.bfloat16, tag="xt", bufs=2)
                    for g in range(N_IC // 8):
                        tp = tps.tile([128, 8 * 128], dt.bfloat16, tag="xtp")
                        for k in range(8):
                            c = g * 8 + k
                            nc.tensor.transpose(tp[:, k * 128:(k + 1) * 128],
                                                xbf[:, c * 128:(c + 1) * 128], ident)
                        nc.scalar.copy(xt[:, g * 8 * 128:(g + 1) * 8 * 128], tp)
                    return xt

                def finish(it, xt):
                    yps = mps.tile([128, O], dt.float32, tag="ypsum")
                    for c in range(N_IC):
                        nc.tensor.matmul(yps, xt[:, c * 128:(c + 1) * 128],
                                         wt[:, c * O:(c + 1) * O],
                                         start=(c == 0), stop=(c == N_IC - 1))
                    yo = mp.tile([128, O], dt.float32, tag="yo")
                    nc.scalar.copy(yo, yps)
                    nc.sync.dma_start(out=y_d[it * 128:(it + 1) * 128, :], in_=yo)

                cur = stage(0)
                for it in range(T // 128):
                    nxt = stage(it + 1) if it + 1 < T // 128 else None
                    finish(it, cur)
                    cur = nxt
    return nc


_CACHE = {}


def _get_runner():
    if "r" in _CACHE:
        return _CACHE["r"]
    nc = bacc.Bacc("TRN2", target_bir_lowering=False, debug=False)
    _build(nc)
    nc.compile()

    import jax
    from jax.experimental.shard_map import shard_map
    from jax.sharding import Mesh, PartitionSpec, NamedSharding
    from concourse.bass2jax import _bass_exec_p, partition_id_tensor, install_neuronx_cc_hook

    install_neuronx_cc_hook()
    in_names, out_names, out_avals = [], [], []
    partition_name = nc.partition_id_tensor.name if nc.partition_id_tensor else None
    for alloc in nc.m.functions[0].allocations:
        if not isinstance(alloc, mybir.MemoryLocationSet):
            continue
        name = alloc.memorylocations[0].name
        if alloc.kind == "ExternalInput":
            if name != partition_name:
                in_names.append(name)
        elif alloc.kind == "ExternalOutput":
            out_names.append(name)
            out_avals.append(jax.core.ShapedArray(tuple(alloc.tensor_shape),
                                                  mybir.dt.np(alloc.dtype)))
    n_params = len(in_names)
    all_in_names = list(in_names) + list(out_names)
    if partition_name is not None:
        all_in_names.append(partition_name)

    def _body(*args):
        operands = list(args)
        if partition_name is not None:
            operands.append(partition_id_tensor())
        return tuple(_bass_exec_p.bind(
            *operands,
            out_avals=tuple(out_avals),
            in_names=tuple(all_in_names),
            out_names=tuple(out_names),
            lowering_input_output_aliases=(),
            sim_require_finite=True,
            sim_require_nnan=True,
            nc=nc,
        ))

    devices = jax.devices()[:N_CORES]
    mesh = Mesh(np.asarray(devices), ("core",))
    n_outs = len(out_avals)
    fn = jax.jit(
        shard_map(_body, mesh=mesh,
                  in_specs=(PartitionSpec("core"),) * (n_params + n_outs),
                  out_specs=(PartitionSpec("core"),) * n_outs,
                  check_rep=False),
        donate_argnums=tuple(range(n_params, n_params + n_outs)),
        keep_unused=True)
    sharding = NamedSharding(mesh, PartitionSpec("core"))
    _CACHE["r"] = (fn, in_names, out_names, out_avals, sharding)
    return _CACHE["r"]


def kernel(x, codes, scales, lora_A, lora_B):
    import jax
    fn, in_names, out_names, out_avals, sharding = _get_runner()

    x = np.ascontiguousarray(x, dtype=np.float32)
    codes = np.ascontiguousarray(codes, dtype=np.int32)
    scales2 = np.ascontiguousarray(scales, dtype=np.float32).reshape(OUT, IN // BLK)
    lora_A = np.ascontiguousarray(lora_A, dtype=np.float32)
    lora_B = np.ascontiguousarray(lora_B, dtype=np.float32)

    per_core = {
        "x": [x] * N_CORES,
        "codes": [codes[c * O:(c + 1) * O] for c in range(N_CORES)],
        "scales": [scales2[c * O:(c + 1) * O] for c in range(N_CORES)],
        "lora_a": [lora_A] * N_CORES,
        "lora_b": [lora_B[c * O:(c + 1) * O] for c in range(N_CORES)],
    }
    concat_in = [np.concatenate(per_core[n], axis=0) for n in in_names]
    dev_in = [jax.device_put(a, sharding) for a in concat_in]
    zeros = [jax.device_put(
        np.zeros((N_CORES * av.shape[0], *av.shape[1:]), av.dtype), sharding)
        for av in out_avals]
    outs = fn(*dev_in, *zeros)
    y_all = np.asarray(outs[out_names.index("y")])  # [8*4096, 512]
    y_shards = y_all.reshape(N_CORES, T, O)
    return np.concatenate([y_shards[c] for c in range(N_CORES)], axis=1)
